# revision 1
# baseline (speedup 1.0000x reference)
"""Causal bilinear self-attention kernel for Trainium2 (8 NeuronCores).

Problem (per reference):
    h: (2, 2048, 512) f32, A: (8, 512, 512) f32
    scores = einsum('btd,hde,bse->bhts', h, A, h); causal mask; softmax
    out = einsum('bhts,bsd->bhtd', attn, h)  -> reshape (2, 2048, 8*512)

Sharding: tensor-parallel over heads — core i computes head i entirely
(no collectives). Each core receives the full h (plus a host-side
transposed copy hT for the matmul layouts) and its own A slice.

Per-core device kernel (per batch b, per 128-row query tile i):
    qT[e,t]   = sum_d A[d,e] h[t,d]          (PE, lhsT=A chunk, rhs=hT)
    S[t,s]    = sum_e qT[e,t] hT[e,s]        (PE, causal s-chunks only)
    softmax along s (free axis): DVE causal mask + chunk maxes on PSUM,
    ACT exp (+fused row sums) PSUM->SBUF, DVE reciprocal;
    normalization is folded into the output scale.
    attnT     = PE transpose of 128x128 attn blocks (via identity)
    out[t,d]  = sum_s attnT[s,t].T h[s,d]    (PE, accumulate in PSUM)
"""

import os
import sys

for _p in ("/opt/trn_rl_repo", "/root/.axon_site/_ro/trn_rl_repo"):
    if os.path.isdir(_p) and _p not in sys.path:
        sys.path.insert(0, _p)

import numpy as np

import concourse.bass as bass
import concourse.mybir as mybir
import concourse.tile as tile
from concourse import bacc
from concourse.bass_utils import run_bass_kernel_spmd

B, T, D, HEADS = 2, 2048, 512, 8
P = 128                 # partition dim / t-tile rows
NT = T // P             # 16 query tiles per batch
SC = 512                # score chunk width (PSUM bank)
NSC = T // SC           # 4 chunks per full score row
KC = D // P             # 4 contraction chunks of 128
MASKVAL = -1.0e30
FP32 = mybir.dt.float32


def build_nc():
    nc = bacc.Bacc("TRN2", debug=False)

    h_d = nc.dram_tensor("h", [B, T, D], FP32, kind="ExternalInput").ap()
    hT_d = nc.dram_tensor("hT", [B, D, T], FP32, kind="ExternalInput").ap()
    A_d = nc.dram_tensor("A", [D, D], FP32, kind="ExternalInput").ap()
    ident_d = nc.dram_tensor("ident", [P, P], FP32, kind="ExternalInput").ap()
    causal_d = nc.dram_tensor("causal", [P, P], FP32, kind="ExternalInput").ap()
    out_d = nc.dram_tensor("out", [B, T, D], FP32, kind="ExternalOutput").ap()

    with tile.TileContext(nc) as tc:
        with (
            tc.tile_pool(name="const", bufs=1) as const_pool,
            tc.tile_pool(name="hsb", bufs=2) as h_pool,
            tc.tile_pool(name="hTsb", bufs=2) as hT_pool,
            tc.tile_pool(name="qTsb", bufs=2) as qT_pool,
            tc.tile_pool(name="attn", bufs=3) as attn_pool,
            tc.tile_pool(name="attnT", bufs=3) as attnT_pool,
            tc.tile_pool(name="osb", bufs=3) as osb_pool,
            tc.tile_pool(name="stat", bufs=8) as stat_pool,
            tc.tile_pool(name="ps_sc", bufs=5, space="PSUM") as ps_sc,
            tc.tile_pool(name="ps_tr", bufs=2, space="PSUM") as ps_tr,
            tc.tile_pool(name="ps_out", bufs=1, space="PSUM") as ps_out,
        ):
            ident = const_pool.tile([P, P], FP32)
            nc.sync.dma_start(ident, ident_d)
            # additive causal mask for the 128x128 diagonal block
            causal = const_pool.tile([P, P], FP32)
            nc.sync.dma_start(causal, causal_d)

            A_sb = const_pool.tile([P, KC, D], FP32)
            nc.sync.dma_start(A_sb, A_d.rearrange("(c p) e -> p c e", p=P))

            for b in range(B):
                h_sb = h_pool.tile([P, NT, D], FP32, tag="hsb")
                for n4 in range(4):
                    nc.sync.dma_start(
                        h_sb[:, 4 * n4:4 * n4 + 4, :],
                        h_d[b, n4 * 512:(n4 + 1) * 512, :].rearrange(
                            "(n p) d -> p n d", p=P),
                    )
                hT_sb = hT_pool.tile([P, KC, T], FP32, tag="hTsb")
                for c in range(KC):
                    nc.sync.dma_start(hT_sb[:, c, :], hT_d[b, c * P:(c + 1) * P, :])

                for tcx in range(NSC):
                    # qT for this 512-wide t range, all 4 e-chunks
                    qT_sb = qT_pool.tile([P, KC, SC], FP32, tag="qTsb")
                    for k in range(KC):
                        q_ps = ps_sc.tile([P, SC], FP32, tag="ps_sc")
                        for m in range(KC):
                            nc.tensor.matmul(
                                q_ps,
                                lhsT=A_sb[:, m, k * P:(k + 1) * P],
                                rhs=hT_sb[:, m, tcx * SC:(tcx + 1) * SC],
                                start=(m == 0),
                                stop=(m == KC - 1),
                            )
                        nc.vector.tensor_copy(out=qT_sb[:, k, :], in_=q_ps)

                    for ii in range(4):
                        i = 4 * tcx + ii        # global query-tile index
                        nch = tcx + 1           # causal 512-chunks incl. diagonal
                        dw = (ii + 1) * P       # valid width of diagonal chunk

                        # scores S[t, s] for s <= t (by chunk); diagonal
                        # 128-block gets the additive causal mask in place
                        sc_sb = []
                        for c in range(nch):
                            w = SC if c < tcx else dw
                            s_ps = ps_sc.tile([P, SC], FP32, tag="ps_sc")
                            for k in range(KC):
                                nc.tensor.matmul(
                                    s_ps[:, :w],
                                    lhsT=qT_sb[:, k, ii * P:(ii + 1) * P],
                                    rhs=hT_sb[:, k, c * SC:c * SC + w],
                                    start=(k == 0),
                                    stop=(k == KC - 1),
                                )
                            if c == nch - 1:
                                nc.vector.tensor_tensor(
                                    out=s_ps[:, w - P:w],
                                    in0=s_ps[:, w - P:w],
                                    in1=causal,
                                    op=mybir.AluOpType.add,
                                )
                            sc_sb.append(s_ps)

                        # row max (per chunk, then combined, negated)
                        mx = stat_pool.tile([P, NSC], FP32, tag="mx")
                        for c in range(nch):
                            w = SC if c < tcx else dw
                            nc.vector.tensor_reduce(
                                out=mx[:, c:c + 1],
                                in_=sc_sb[c][:, :w],
                                axis=mybir.AxisListType.X,
                                op=mybir.AluOpType.max,
                            )
                        negmax = stat_pool.tile([P, 1], FP32, tag="negmax")
                        nc.vector.tensor_reduce(
                            out=negmax,
                            in_=mx[:, :nch],
                            axis=mybir.AxisListType.X,
                            op=mybir.AluOpType.max,
                            negate=True,
                        )

                        # attn = exp(S - max), row sums fused into the ACT pass
                        attn = attn_pool.tile([P, T], FP32, tag="attn")
                        sums = stat_pool.tile([P, NSC], FP32, tag="sums")
                        for c in range(nch):
                            w = SC if c < tcx else dw
                            nc.scalar.activation(
                                out=attn[:, c * SC:c * SC + w],
                                in_=sc_sb[c][:, :w],
                                func=mybir.ActivationFunctionType.Exp,
                                bias=negmax,
                                scale=1.0,
                                accum_out=sums[:, c:c + 1],
                            )
                        tot = stat_pool.tile([P, 1], FP32, tag="tot")
                        nc.vector.tensor_reduce(
                            out=tot,
                            in_=sums[:, :nch],
                            axis=mybir.AxisListType.X,
                            op=mybir.AluOpType.add,
                        )
                        recip = stat_pool.tile([P, 1], FP32, tag="recip")
                        nc.vector.reciprocal(recip, tot)

                        # transpose attn blocks (PE) then copy PSUM->SBUF (DVE)
                        nblk = i + 1
                        aT_tiles = []
                        for g in range((nblk + 3) // 4):
                            jlo = 4 * g
                            jhi = min(nblk, jlo + 4)
                            tr_ps = ps_tr.tile([P, SC], FP32, tag="ps_tr")
                            for j in range(jlo, jhi):
                                nc.tensor.transpose(
                                    tr_ps[:, (j - jlo) * P:(j - jlo + 1) * P],
                                    attn[:, j * P:(j + 1) * P],
                                    ident,
                                )
                            aT = attnT_pool.tile([P, SC], FP32, tag="attnT")
                            nc.vector.tensor_copy(
                                out=aT[:, :(jhi - jlo) * P],
                                in_=tr_ps[:, :(jhi - jlo) * P],
                            )
                            aT_tiles.append(aT)

                        # out[t, :] = sum_s attn[t, s] h[s, :]
                        o_ps = ps_out.tile([P, D], FP32, tag="ps_out")
                        for j in range(nblk):
                            aT = aT_tiles[j // 4]
                            nc.tensor.matmul(
                                o_ps,
                                lhsT=aT[:, (j % 4) * P:(j % 4 + 1) * P],
                                rhs=h_sb[:, j, :],
                                start=(j == 0),
                                stop=(j == nblk - 1),
                            )

                        osb = osb_pool.tile([P, D], FP32, tag="osb")
                        nc.vector.tensor_scalar_mul(osb, o_ps, recip)
                        nc.sync.dma_start(out_d[b, i * P:(i + 1) * P, :], osb)

    nc.compile()
    return nc


_CACHE: dict = {}


def kernel(h: np.ndarray, A: np.ndarray) -> np.ndarray:
    if "nc" not in _CACHE:
        _CACHE["nc"] = build_nc()
    nc = _CACHE["nc"]

    h32 = np.ascontiguousarray(h, dtype=np.float32)
    hT = np.ascontiguousarray(h32.transpose(0, 2, 1))
    ident_np = np.eye(P, dtype=np.float32)
    causal_np = np.where(
        np.arange(P)[:, None] >= np.arange(P)[None, :], 0.0, MASKVAL
    ).astype(np.float32)
    in_maps = [
        {"h": h32, "hT": hT, "A": np.ascontiguousarray(A[i], dtype=np.float32),
         "ident": ident_np, "causal": causal_np}
        for i in range(HEADS)
    ]
    res = run_bass_kernel_spmd(nc, in_maps, core_ids=list(range(HEADS)))
    out = np.stack([res.results[i]["out"] for i in range(HEADS)], axis=1)
    # (B, heads, T, d) -> raw row-major reshape, matching the reference's
    # torch-style .view(B, T, heads*d) on a contiguous (B, heads, T, d)
    return np.ascontiguousarray(out.reshape(B, T, HEADS * D))



# revision 2
# speedup vs baseline: 3.0105x; 3.0105x over previous
"""Causal bilinear self-attention kernel for Trainium2 (8 NeuronCores).

Problem (per reference):
    h: (2, 2048, 512) f32, A: (8, 512, 512) f32
    scores = einsum('btd,hde,bse->bhts', h, A, h); causal mask; softmax
    out = einsum('bhts,bsd->bhtd', attn, h)  -> reshape (2, 2048, 8*512)

Sharding: tensor-parallel over heads - core i computes head i entirely
(no collectives).

Precision/performance scheme (PE matmul cost is N cycles/row at bf16 or
fp32r with N>=256; plain fp32 costs 4 N):
  - qT = A^T h^T and scores S = qT^T hT run in float32r (fp32 rounded to
    11 mantissa bits; PE computes exact products of rounded inputs at
    full bf16 rate). Host pre-rounds h^T and A to fp32r; the qT
    PSUM->SBUF copy rounds on the DVE.
  - Score chunks are decomposed into widths of {256,384,512} so every
    fp32r matmul has moving free-size >= 256 (below 256 fp32r drops to
    1/4 rate).
  - softmax: DVE per-chunk maxes on PSUM, ACT exp (bias=-rowmax, fused
    row-sum accumulation) writing bf16 attn; DVE reciprocal. The
    normalization is folded into the output scale.
  - attn blocks are PE-transposed in bf16 (1 cyc/row) and out = attnT^T
    h accumulates in fp32 PSUM with bf16 operands (h supplied as bf16
    from the host).
  End-to-end rel err vs the fp32 reference ~2e-3 (dominated by fp32r
  score rounding; gate is 2e-2).
"""

import os
import sys

for _p in ("/opt/trn_rl_repo", "/root/.axon_site/_ro/trn_rl_repo"):
    if os.path.isdir(_p) and _p not in sys.path:
        sys.path.insert(0, _p)

import numpy as np

import concourse.bass as bass
import concourse.mybir as mybir
import concourse.tile as tile
from concourse import bacc
from concourse.bass_utils import run_bass_kernel_spmd

B, T, D, HEADS = 2, 2048, 512, 8
P = 128                 # partition dim / t-tile rows
NT = T // P             # 16 query tiles per batch
SC = 512                # max score chunk width (PSUM bank)
NSC = T // SC           # 4 max chunks per full score row
KC = D // P             # 4 contraction chunks of 128
MASKVAL = -1.0e30
FP32 = mybir.dt.float32
FP32R = mybir.dt.float32r
BF16 = mybir.dt.bfloat16


def round_fp32r(x: np.ndarray) -> np.ndarray:
    """Round fp32 to fp32r (11 explicit mantissa bits, RNE)."""
    u = np.ascontiguousarray(x, dtype=np.float32).view(np.uint32).astype(np.uint64)
    u = (u + 0x7FF + ((u >> 12) & 1)) & np.uint64(0xFFFFF000)
    return u.astype(np.uint32).view(np.float32)


def _pieces(nblk: int) -> list:
    """Decompose a causal width of `nblk` 128-blocks into score-chunk
    widths of {2,3,4} blocks (>=256 cols each, <=512 for one PSUM bank).
    nblk==1 rounds up to 2 blocks; cols 128:256 are fully masked."""
    if nblk == 1:
        return [2]
    out = []
    while nblk > 5:
        out.append(4)
        nblk -= 4
    if nblk == 5:
        out.extend([3, 2])
    else:
        out.append(nblk)
    return out


def build_nc():
    nc = bacc.Bacc("TRN2", debug=False)

    hT_d = nc.dram_tensor("hT", [B, D, T], FP32R, kind="ExternalInput").ap()
    hb_d = nc.dram_tensor("hb", [B, T, D], BF16, kind="ExternalInput").ap()
    A_d = nc.dram_tensor("A", [D, D], FP32R, kind="ExternalInput").ap()
    ident_d = nc.dram_tensor("ident", [P, P], BF16, kind="ExternalInput").ap()
    causal_d = nc.dram_tensor("causal", [P, 2 * P], FP32, kind="ExternalInput").ap()
    out_d = nc.dram_tensor("out", [B, T, D], FP32, kind="ExternalOutput").ap()

    with tile.TileContext(nc) as tc:
        with (
            tc.tile_pool(name="const", bufs=1) as const_pool,
            tc.tile_pool(name="hbsb", bufs=2) as hb_pool,
            tc.tile_pool(name="hTsb", bufs=2) as hT_pool,
            tc.tile_pool(name="qTsb", bufs=2) as qT_pool,
            tc.tile_pool(name="attn", bufs=3) as attn_pool,
            tc.tile_pool(name="attnT", bufs=3) as attnT_pool,
            tc.tile_pool(name="osb", bufs=3) as osb_pool,
            tc.tile_pool(name="stat", bufs=8) as stat_pool,
            tc.tile_pool(name="ps_sc", bufs=5, space="PSUM") as ps_sc,
            tc.tile_pool(name="ps_tr", bufs=2, space="PSUM") as ps_tr,
            tc.tile_pool(name="ps_out", bufs=1, space="PSUM") as ps_out,
        ):
            ident = const_pool.tile([P, P], BF16)
            nc.sync.dma_start(ident, ident_d)
            # additive causal mask: cols 0:128 triangular for the
            # diagonal 128-block, cols 128:256 fully masked (tile 0 pad)
            causal = const_pool.tile([P, 2 * P], FP32)
            nc.sync.dma_start(causal, causal_d)

            A_sb = const_pool.tile([P, KC, D], FP32R)
            nc.sync.dma_start(A_sb, A_d.rearrange("(c p) e -> p c e", p=P))

            for b in range(B):
                hb_sb = hb_pool.tile([P, NT, D], BF16, tag="hbsb")
                for n4 in range(4):
                    nc.sync.dma_start(
                        hb_sb[:, 4 * n4:4 * n4 + 4, :],
                        hb_d[b, n4 * 512:(n4 + 1) * 512, :].rearrange(
                            "(n p) d -> p n d", p=P),
                    )
                hT_sb = hT_pool.tile([P, KC, T], FP32R, tag="hTsb")
                for c in range(KC):
                    nc.sync.dma_start(hT_sb[:, c, :], hT_d[b, c * P:(c + 1) * P, :])

                for tcx in range(NSC):
                    # qT for this 512-wide t range, all 4 e-chunks
                    qT_sb = qT_pool.tile([P, KC, SC], FP32R, tag="qTsb")
                    for k in range(KC):
                        q_ps = ps_sc.tile([P, SC], FP32, tag="ps_sc")
                        for m in range(KC):
                            nc.tensor.matmul(
                                q_ps,
                                lhsT=A_sb[:, m, k * P:(k + 1) * P],
                                rhs=hT_sb[:, m, tcx * SC:(tcx + 1) * SC],
                                start=(m == 0),
                                stop=(m == KC - 1),
                            )
                        nc.vector.tensor_copy(out=qT_sb[:, k, :], in_=q_ps)

                    for ii in range(4):
                        i = 4 * tcx + ii        # global query-tile index
                        pieces = _pieces(i + 1)
                        npc = len(pieces)

                        # scores S[t, s] by >=256-wide chunks; the final
                        # chunk ends at the diagonal and gets the
                        # additive causal mask on its last 128 cols
                        # (256 cols for tile 0, whose width is padded)
                        sc_ps = []
                        off = 0
                        for c, wblk in enumerate(pieces):
                            w = wblk * P
                            s_ps = ps_sc.tile([P, SC], FP32, tag="ps_sc")
                            for k in range(KC):
                                nc.tensor.matmul(
                                    s_ps[:, :w],
                                    lhsT=qT_sb[:, k, ii * P:(ii + 1) * P],
                                    rhs=hT_sb[:, k, off:off + w],
                                    start=(k == 0),
                                    stop=(k == KC - 1),
                                )
                            if c == npc - 1:
                                mw = 2 * P if i == 0 else P
                                nc.vector.tensor_tensor(
                                    out=s_ps[:, w - mw:w],
                                    in0=s_ps[:, w - mw:w],
                                    in1=causal[:, :mw],
                                    op=mybir.AluOpType.add,
                                )
                            sc_ps.append((s_ps, off, w))
                            off += w

                        # row max (per chunk, then combined, negated)
                        mx = stat_pool.tile([P, NSC], FP32, tag="mx")
                        for c, (s_ps, _, w) in enumerate(sc_ps):
                            nc.vector.tensor_reduce(
                                out=mx[:, c:c + 1],
                                in_=s_ps[:, :w],
                                axis=mybir.AxisListType.X,
                                op=mybir.AluOpType.max,
                            )
                        negmax = stat_pool.tile([P, 1], FP32, tag="negmax")
                        nc.vector.tensor_reduce(
                            out=negmax,
                            in_=mx[:, :npc],
                            axis=mybir.AxisListType.X,
                            op=mybir.AluOpType.max,
                            negate=True,
                        )

                        # attn = exp(S - max) in bf16, row sums fused
                        attn = attn_pool.tile([P, T], BF16, tag="attn")
                        sums = stat_pool.tile([P, NSC], FP32, tag="sums")
                        for c, (s_ps, coff, w) in enumerate(sc_ps):
                            nc.scalar.activation(
                                out=attn[:, coff:coff + w],
                                in_=s_ps[:, :w],
                                func=mybir.ActivationFunctionType.Exp,
                                bias=negmax,
                                scale=1.0,
                                accum_out=sums[:, c:c + 1],
                            )
                        tot = stat_pool.tile([P, 1], FP32, tag="tot")
                        nc.vector.tensor_reduce(
                            out=tot,
                            in_=sums[:, :npc],
                            axis=mybir.AxisListType.X,
                            op=mybir.AluOpType.add,
                        )
                        recip = stat_pool.tile([P, 1], FP32, tag="recip")
                        nc.vector.reciprocal(recip, tot)

                        # transpose attn blocks (PE, bf16) then copy
                        # PSUM->SBUF (DVE)
                        nblk = i + 1
                        aT_tiles = []
                        for g in range((nblk + 3) // 4):
                            jlo = 4 * g
                            jhi = min(nblk, jlo + 4)
                            tr_ps = ps_tr.tile([P, SC], BF16, tag="ps_tr")
                            for j in range(jlo, jhi):
                                nc.tensor.transpose(
                                    tr_ps[:, (j - jlo) * P:(j - jlo + 1) * P],
                                    attn[:, j * P:(j + 1) * P],
                                    ident,
                                )
                            aT = attnT_pool.tile([P, SC], BF16, tag="attnT")
                            nc.vector.tensor_copy(
                                out=aT[:, :(jhi - jlo) * P],
                                in_=tr_ps[:, :(jhi - jlo) * P],
                            )
                            aT_tiles.append(aT)

                        # out[t, :] = sum_s attn[t, s] h[s, :]  (bf16)
                        o_ps = ps_out.tile([P, D], FP32, tag="ps_out")
                        for j in range(nblk):
                            aT = aT_tiles[j // 4]
                            nc.tensor.matmul(
                                o_ps,
                                lhsT=aT[:, (j % 4) * P:(j % 4 + 1) * P],
                                rhs=hb_sb[:, j, :],
                                start=(j == 0),
                                stop=(j == nblk - 1),
                            )

                        osb = osb_pool.tile([P, D], FP32, tag="osb")
                        nc.vector.tensor_scalar_mul(osb, o_ps, recip)
                        nc.sync.dma_start(out_d[b, i * P:(i + 1) * P, :], osb)

    nc.compile()
    return nc


def _make_in_maps(h: np.ndarray, A: np.ndarray) -> list:
    import ml_dtypes

    h32 = np.ascontiguousarray(h, dtype=np.float32)
    hT = round_fp32r(h32.transpose(0, 2, 1))
    hb = h32.astype(ml_dtypes.bfloat16)
    ident_np = np.eye(P, dtype=ml_dtypes.bfloat16)
    causal_np = np.full((P, 2 * P), MASKVAL, dtype=np.float32)
    tri = np.arange(P)[:, None] >= np.arange(P)[None, :]
    causal_np[:, :P] = np.where(tri, 0.0, MASKVAL)
    return [
        {"hT": hT, "hb": hb, "A": round_fp32r(A[i]),
         "ident": ident_np, "causal": causal_np}
        for i in range(A.shape[0])
    ]


_CACHE: dict = {}


def kernel(h: np.ndarray, A: np.ndarray) -> np.ndarray:
    if "nc" not in _CACHE:
        _CACHE["nc"] = build_nc()
    nc = _CACHE["nc"]

    in_maps = _make_in_maps(h, A)
    res = run_bass_kernel_spmd(nc, in_maps, core_ids=list(range(HEADS)))
    out = np.stack([res.results[i]["out"] for i in range(HEADS)], axis=1)
    # (B, heads, T, d) -> raw row-major reshape, matching the reference's
    # torch-style .view(B, T, heads*d) on a contiguous (B, heads, T, d)
    return np.ascontiguousarray(out.reshape(B, T, HEADS * D))
